# revision 10
# baseline (speedup 1.0000x reference)
"""Multi-head self-attention on 8 TRN2 NeuronCores.

Sharding: core c -> (batch b = c//2, head-half g = c%2, i.e. 8 of 16 heads).
Each core computes qkv-proj + attention + out-proj partial for its 8 heads;
host sums the two partials per batch and adds b_out.

Design (v2):
- stage1 q,k,v projections in fp16 (1 cyc/row), q/k results quantized to
  fp8e4 on the PSUM->SBUF copy (bias added via per-partition tensor_scalar).
- scores as zero-padded DoubleRow fp8 matmuls (0.5 cyc/row): operands
  [64, 2, *] with the i=1 plane zeroed; out tile [128 k-pos, 512 q-pos].
- exp on ACT (scale=0.125 applied in the activation), fp16 out.
- ctx computed transposed: out [128 q, 65] with lhsT = exp tile (stationary)
  and rhs = V' [128 k, 65] whose 65th column is ones -> denominator lands in
  out[:, 64] = per-partition scalar. Normalization + V-bias is then a single
  scalar_tensor_tensor (mult, add) per (head, q-tile).
- ctx^T via PE transpose (fp16, identity rhs) packing head pairs into
  [128, 128] PSUM tiles; out-projection over the packed [d, q] layout,
  fp16 output DMA'd per tile; host sums core pairs + b_out.
- software pipelining: unit (pair, qc) emits its 32 score matmuls + 16 exps,
  then the previous unit's ctx/norm/transpose tail, then next-pair stage1
  or out-projection work, keeping ACT (the bottleneck) saturated.
"""
import sys
sys.path.insert(0, '/opt/trn_rl_repo')

import numpy as np

import concourse.bass as bass
import concourse.mybir as mybir
import concourse.tile as tile
from concourse import bacc

F32 = mybir.dt.float32
F16 = mybir.dt.float16
F8E4 = mybir.dt.float8e4
DR = mybir.MatmulPerfMode.DoubleRow
Exp = mybir.ActivationFunctionType.Exp
MULT = mybir.AluOpType.mult
ADD = mybir.AluOpType.add

B, S, D = 4, 2048, 1024
H, HD = 16, 64
N_CORES = 8
NSK = S // 128            # 16 k-chunks of 128
NQT = S // 128            # 16 q-tiles of 128
SCALE = 0.125             # 1/sqrt(HD)


def build_nc(skip_tail=False, skip_exp=False, sc_bufs=2, skip_v=False, fake_in=False):
    nc = bacc.Bacc(None, target_bir_lowering=False)

    x16_d = nc.dram_tensor("x16", [128, 8, S], F16, kind="ExternalInput")
    wqk_d = nc.dram_tensor("wqk", [128, 8, 1024], F16, kind="ExternalInput")
    wv_d = nc.dram_tensor("wv", [128, 8, 512], F16, kind="ExternalInput")
    wout_d = nc.dram_tensor("wout", [128, 4, D], F16, kind="ExternalInput")
    bqk_d = nc.dram_tensor("bqk", [128, 8], F32, kind="ExternalInput")
    bv_d = nc.dram_tensor("bv", [128, 8, HD], F16, kind="ExternalInput")
    ident_d = nc.dram_tensor("ident", [128, 128], F16, kind="ExternalInput")
    out_d = nc.dram_tensor("out", [S, D], F16, kind="ExternalOutput")

    with tile.TileContext(nc) as tc:
        with (
            tc.tile_pool(name="const", bufs=1) as cpool,
            tc.tile_pool(name="expT", bufs=4) as expT_pool,
            tc.tile_pool(name="ctxN", bufs=4) as ctxN_pool,
            tc.tile_pool(name="rcp", bufs=4) as rcp_pool,
            tc.tile_pool(name="osb", bufs=2) as out_pool,
            tc.tile_pool(name="scps", bufs=sc_bufs, space="PSUM") as sc_ps,
            tc.tile_pool(name="ctxps", bufs=2, space="PSUM") as ctx_ps,
            tc.tile_pool(name="shps", bufs=2, space="PSUM") as sh_ps,
        ):
            # ---- constants / persistent tiles ----
            x16 = cpool.tile([128, 8, S], F16)
            wqk = cpool.tile([128, 8, 1024], F16)
            if fake_in:
                nc.vector.memset(x16[:], 0.01)
                nc.vector.memset(wqk[:], 0.01)
            else:
                for n in range(4):
                    nc.sync.dma_start(x16[:, :, 512 * n:512 * (n + 1)],
                                      x16_d[:, :, 512 * n:512 * (n + 1)])
                nc.sync.dma_start(wqk[:], wqk_d[:])
            wv = cpool.tile([128, 8, 512], F16)
            nc.sync.dma_start(wv[:], wv_d[:])
            wout = cpool.tile([128, 4, D], F16)
            nc.sync.dma_start(wout[:], wout_d[:])
            bqk = cpool.tile([128, 8], F32)
            nc.sync.dma_start(bqk[:], bqk_d[:])
            bv = cpool.tile([128, 8, HD], F16)
            nc.sync.dma_start(bv[:], bv_d[:])
            ident = cpool.tile([128, 128], F16)
            nc.sync.dma_start(ident[:], ident_d[:])

            # V': [s%128, sk, head, hd+1]; [..,64] = 1.0 for denominators
            v_sb = cpool.tile([128, NSK, 8, HD + 1], F16)
            nc.vector.memset(v_sb[:, :, :, HD], 1.0)

            # q/k fp8 double-buffered (ping-pong by pair parity).
            # dims: [part(=hd within head pair), qk, i(double-row), s]
            qk8 = [cpool.tile([128, 2, 2, S], F8E4, name=f"qk8_{b_}")
                   for b_ in range(2)]
            for b_ in range(2):
                nc.vector.memset(qk8[b_][:, :, 1, :], 0.0)

            # ctx^T accumulator [d-part packed by pair, pair, q] fp16
            ctxT = cpool.tile([128, 4, S], F16)

            # ---- stage1 helpers ----
            def s1_qk_tile(p, j, n):
                """pair p, j=0 q / j=1 k, seq chunk n -> qk8[p%2][:, j, 0, ...]."""
                ps = sh_ps.tile([128, 512], F32, name="s1", tag="sh")
                foff = 128 * p + 512 * j
                for kc in range(8):
                    nc.tensor.matmul(
                        ps[:], wqk[:, kc, foff:foff + 128],
                        x16[:, kc, 512 * n:512 * (n + 1)],
                        start=(kc == 0), stop=(kc == 7))
                nc.vector.tensor_scalar_add(
                    qk8[p % 2][:, j, 0, 512 * n:512 * (n + 1)], ps[:],
                    bqk[:, 4 * j + p:4 * j + p + 1])

            def s1_v_tile(p, t):
                """v for pair p (128 feats), seq tile t (128 rows)."""
                ps = sh_ps.tile([128, 128], F32, name="s1v", tag="sh")
                for kc in range(8):
                    nc.tensor.matmul(
                        ps[:], x16[:, kc, 128 * t:128 * (t + 1)],
                        wv[:, kc, 128 * p:128 * (p + 1)],
                        start=(kc == 0), stop=(kc == 7))
                nc.vector.tensor_copy(
                    v_sb[:, t, 2 * p:2 * p + 2, 0:HD],
                    ps.rearrange("a (h d) -> a h d", h=2))

            # ---- deferred PE work queue: popped between score/exp pairs so
            # the ACT engine (bottleneck) never starves while PE does the
            # ctx/transpose/stage1/outproj work of earlier units ----
            from collections import deque
            work_q = deque()

            def pop_work(k):
                for _ in range(k):
                    if work_q:
                        work_q.popleft()()

            def push_tail(p, qc, expTs):
                """Queue the consumer tail of unit (p, qc)."""
                if skip_tail:
                    return
                def chain(hi, qt, cn):
                    def run():
                        h = 2 * p + hi
                        cps = ctx_ps.tile([128, HD + 1], F32,
                                          name="ctx", tag="ctx")
                        for sk in range(NSK):
                            nc.tensor.matmul(
                                cps[:],
                                expTs[hi][:, sk, 128 * qt:128 * (qt + 1)],
                                v_sb[:, sk, h, :],
                                start=(sk == 0), stop=(sk == NSK - 1))
                        rcp = rcp_pool.tile([128, 1], F32, name="rc",
                                            tag="rc")
                        nc.vector.reciprocal_approx_fast(
                            rcp[:], cps[:, HD:HD + 1])
                        nc.vector.scalar_tensor_tensor(
                            cn[:, hi, :], cps[:, 0:HD], rcp[:], bv[:, h, :],
                            op0=MULT, op1=ADD)
                    return run

                def transp(qt, cn):
                    def run():
                        tp = sh_ps.tile([128, 128], F16, name="tp", tag="sh")
                        nc.tensor.matmul(
                            tp[0:64, :], cn[:, 0, :], ident[:],
                            start=True, stop=True, is_transpose=True,
                            tile_position=(0, 0))
                        nc.tensor.matmul(
                            tp[64:128, :], cn[:, 1, :], ident[:],
                            start=True, stop=True, is_transpose=True,
                            tile_position=(0, 64))
                        qoff = 512 * qc + 128 * qt
                        nc.vector.tensor_copy(
                            ctxT[:, p, qoff:qoff + 128], tp[:])
                    return run

                for qt in range(4):
                    cn = ctxN_pool.tile([128, 2, HD], F16, name="cn",
                                        tag="cn")
                    work_q.append(chain(0, qt, cn))
                    work_q.append(chain(1, qt, cn))
                    work_q.append(transp(qt, cn))

                if p == 3:
                    def oproj(qt, dc):
                        def run():
                            qoff = 512 * qc + 128 * qt
                            ops = sh_ps.tile([128, 512], F32, name="op",
                                             tag="sh")
                            for c in range(4):
                                nc.tensor.matmul(
                                    ops[:], ctxT[:, c, qoff:qoff + 128],
                                    wout[:, c, 512 * dc:512 * (dc + 1)],
                                    start=(c == 0), stop=(c == 3))
                            o16 = out_pool.tile([128, 512], F16, name="o")
                            nc.vector.tensor_copy(o16[:], ops[:])
                            nc.sync.dma_start(
                                out_d[qoff:qoff + 128,
                                      512 * dc:512 * (dc + 1)], o16[:])
                        return run
                    for qt in range(4):
                        for dc in range(2):
                            work_q.append(oproj(qt, dc))

            # ---- prologue: just k chunk 0 + q chunk 0 of pair 0; the rest
            # of pair-0 stage1 and its v-proj ride the work queue ----
            s1_qk_tile(0, 1, 0)
            s1_qk_tile(0, 0, 0)
            for (j, n) in [(1, 1), (1, 2), (1, 3), (0, 1), (0, 2), (0, 3)]:
                work_q.append((lambda j=j, n=n: s1_qk_tile(0, j, n)))
            if not skip_v:
                for t in range(NSK):
                    work_q.append((lambda t=t: s1_v_tile(0, t)))

            # ---- main software-pipelined unit loop ----
            for p in range(4):
                for qc in range(4):
                    u = 4 * p + qc
                    buf = qk8[p % 2]
                    expTs = {}
                    slot = 0
                    for hi in range(2):
                        expTs[hi] = expT_pool.tile([128, NSK, 512], F16,
                                                   name=f"e{hi}", tag="expT")
                        for g in range(8):
                            scp = sc_ps.tile([128, 2, 512], F32, name="sc",
                                             tag="sc")
                            for gg in range(2):
                                sk = 2 * g + gg
                                nc.tensor.matmul(
                                    scp[:, gg, :],
                                    buf[64 * hi:64 * (hi + 1), 1, :,
                                        128 * sk:128 * (sk + 1)],
                                    buf[64 * hi:64 * (hi + 1), 0, :,
                                        512 * qc:512 * (qc + 1)],
                                    start=True, stop=True, perf_mode=DR)
                            if not skip_exp:
                                nc.scalar.activation(
                                    expTs[hi][:, 2 * g:2 * g + 2, :], scp[:],
                                    Exp, scale=SCALE)
                            else:
                                nc.vector.tensor_copy(
                                    expTs[hi][:, 2 * g:2 * g + 2, :], scp[:])
                            pop_work(2 if slot % 3 == 2 else 1)
                            slot += 1

                    # queue work for later units
                    if p < 3 and not skip_v:
                        for t in range(4):
                            work_q.append(
                                (lambda p=p, t=t, qc=qc:
                                 s1_v_tile(p + 1, 4 * qc + t)))
                        jn = [(1, 0), (1, 1)] if qc == 0 else \
                             [(1, 2), (1, 3)] if qc == 1 else \
                             [(0, 0), (0, 1)] if qc == 2 else \
                             [(0, 2), (0, 3)]
                        for (j, n) in jn:
                            work_q.append(
                                (lambda p=p, j=j, n=n:
                                 s1_qk_tile(p + 1, j, n)))
                    push_tail(p, qc, expTs)

            # drain
            while work_q:
                work_q.popleft()()

    nc.compile()
    return nc


# ---------------------------------------------------------------------------
# host side: shard, run SPMD, gather
# ---------------------------------------------------------------------------

_RUNNER = None


def _make_runner(nc, n_cores):
    """Jit-once SPMD runner via PJRT (axon)."""
    import jax
    from jax.sharding import Mesh, PartitionSpec
    from jax.experimental.shard_map import shard_map
    from concourse import bass2jax
    from concourse.bass2jax import _bass_exec_p, install_neuronx_cc_hook

    install_neuronx_cc_hook()
    partition_name = nc.partition_id_tensor.name if nc.partition_id_tensor else None

    in_names, out_names, out_avals, zero_outs = [], [], [], []
    for alloc in nc.m.functions[0].allocations:
        if not isinstance(alloc, mybir.MemoryLocationSet):
            continue
        name = alloc.memorylocations[0].name
        if alloc.kind == "ExternalInput":
            if name != partition_name:
                in_names.append(name)
        elif alloc.kind == "ExternalOutput":
            out_names.append(name)
            shape = tuple(alloc.tensor_shape)
            dtype = mybir.dt.np(alloc.dtype)
            out_avals.append(jax.core.ShapedArray(shape, dtype))
            zero_outs.append(np.zeros(shape, dtype))
    n_params = len(in_names)
    n_outs = len(out_avals)
    all_in_names = list(in_names) + list(out_names)
    if partition_name is not None:
        all_in_names.append(partition_name)

    def _body(*args):
        operands = list(args)
        if partition_name is not None:
            operands.append(bass2jax.partition_id_tensor())
        outs = _bass_exec_p.bind(
            *operands,
            out_avals=tuple(out_avals),
            in_names=tuple(all_in_names),
            out_names=tuple(out_names),
            lowering_input_output_aliases=(),
            sim_require_finite=True,
            sim_require_nnan=True,
            nc=nc,
        )
        return tuple(outs)

    devices = jax.devices()[:n_cores]
    mesh = Mesh(np.asarray(devices), ("core",))
    in_specs = (PartitionSpec("core"),) * (n_params + n_outs)
    out_specs = (PartitionSpec("core"),) * n_outs
    jitted = jax.jit(
        shard_map(_body, mesh=mesh, in_specs=in_specs, out_specs=out_specs,
                  check_rep=False),
        keep_unused=True,
    )

    def run(in_maps):
        concat_in = [
            np.concatenate([np.asarray(in_maps[c][n]) for c in range(n_cores)],
                           axis=0)
            for n in in_names
        ]
        concat_zero = [
            np.zeros((n_cores * z.shape[0], *z.shape[1:]), z.dtype)
            for z in zero_outs
        ]
        out_arrs = jitted(*concat_in, *concat_zero)
        jax.block_until_ready(out_arrs)
        return [
            {n: np.asarray(out_arrs[i]).reshape(n_cores, *out_avals[i].shape)[c]
             for i, n in enumerate(out_names)}
            for c in range(n_cores)
        ]

    return run


def _shard_inputs(qkv, W_in, b_in, W_out, b_out):
    """Build the 8 per-core input dicts."""
    f16 = np.float16
    x = np.asarray(qkv, np.float32)
    W_in = np.asarray(W_in, np.float32)
    b_in = np.asarray(b_in, np.float32)
    W_out = np.asarray(W_out, np.float32)
    ident = np.eye(128, dtype=f16)

    in_maps = []
    for c in range(N_CORES):
        b, g = divmod(c, 2)
        qs = slice(512 * g, 512 * (g + 1))
        ks = slice(1024 + 512 * g, 1024 + 512 * (g + 1))
        vs = slice(2048 + 512 * g, 2048 + 512 * (g + 1))
        xT = np.ascontiguousarray(x[b].T)                     # [D, S]
        # x16[p, kc, s] = xT[128*kc+p, s]
        x16 = xT.reshape(8, 128, S).transpose(1, 0, 2).astype(f16)
        # wqk[p, kc, f]: f 0..511 q feats, 512..1023 k feats
        wq = W_in[:, qs].reshape(8, 128, 512).transpose(1, 0, 2)
        wk = W_in[:, ks].reshape(8, 128, 512).transpose(1, 0, 2)
        wqk = np.concatenate([wq, wk], axis=2).astype(f16)
        wv = W_in[:, vs].reshape(8, 128, 512).transpose(1, 0, 2).astype(f16)
        # wout[p, c_, dout] = W_out[512*g + 128*c_ + p, dout]
        wout = W_out[512 * g:512 * (g + 1), :].reshape(4, 128, D) \
            .transpose(1, 0, 2).astype(f16)
        bqk = np.concatenate([b_in[qs], b_in[ks]]).reshape(8, 128).T \
            .astype(np.float32)
        bqk = np.ascontiguousarray(bqk)
        bv = np.broadcast_to(b_in[vs].reshape(8, HD), (128, 8, HD)) \
            .astype(f16)
        in_maps.append({
            "x16": x16,
            "wqk": wqk,
            "wv": wv,
            "wout": wout,
            "bqk": bqk,
            "bv": np.ascontiguousarray(bv),
            "ident": ident,
        })
    return in_maps


def kernel(qkv, W_in, b_in, W_out, b_out):
    global _RUNNER
    if _RUNNER is None:
        nc = build_nc()
        _RUNNER = _make_runner(nc, N_CORES)
    in_maps = _shard_inputs(qkv, W_in, b_in, W_out, b_out)
    results = _RUNNER(in_maps)
    b_out = np.asarray(b_out, np.float32)
    out = np.empty((B, S, D), np.float32)
    for b in range(B):
        out[b] = (results[2 * b]["out"].astype(np.float32)
                  + results[2 * b + 1]["out"].astype(np.float32) + b_out)
    return out


if __name__ == "__main__":
    rng = np.random.default_rng(0)
    qkv = rng.standard_normal((B, S, D)).astype(np.float32)
    sc = 1.0 / np.sqrt(D)
    W_in = rng.uniform(-sc, sc, (D, 3 * D)).astype(np.float32)
    b_in = rng.uniform(-sc, sc, (3 * D,)).astype(np.float32)
    W_out = rng.uniform(-sc, sc, (D, D)).astype(np.float32)
    b_out = rng.uniform(-sc, sc, (D,)).astype(np.float32)
    got = kernel(qkv, W_in, b_in, W_out, b_out)
    print("kernel ran, output shape", got.shape)


# revision 15
# speedup vs baseline: 1.0446x; 1.0446x over previous
"""Multi-head self-attention on 8 TRN2 NeuronCores.

Sharding: core c -> (batch b = c//2, head-half g = c%2, i.e. 8 of 16 heads).
Each core computes qkv-proj + attention + out-proj partial for its 8 heads;
host sums the two partials per batch and adds b_out.

Design (v2):
- stage1 q,k,v projections in fp16 (1 cyc/row), q/k results quantized to
  fp8e4 on the PSUM->SBUF copy (bias added via per-partition tensor_scalar).
- scores as zero-padded DoubleRow fp8 matmuls (0.5 cyc/row): operands
  [64, 2, *] with the i=1 plane zeroed; out tile [128 k-pos, 512 q-pos].
- exp on ACT (scale=0.125 applied in the activation), fp16 out.
- ctx computed transposed: out [128 q, 65] with lhsT = exp tile (stationary)
  and rhs = V' [128 k, 65] whose 65th column is ones -> denominator lands in
  out[:, 64] = per-partition scalar. Normalization + V-bias is then a single
  scalar_tensor_tensor (mult, add) per (head, q-tile).
- ctx^T via PE transpose (fp16, identity rhs) packing head pairs into
  [128, 128] PSUM tiles; out-projection over the packed [d, q] layout,
  fp16 output DMA'd per tile; host sums core pairs + b_out.
- software pipelining: unit (pair, qc) emits its 32 score matmuls + 16 exps,
  then the previous unit's ctx/norm/transpose tail, then next-pair stage1
  or out-projection work, keeping ACT (the bottleneck) saturated.
"""
import sys
sys.path.insert(0, '/opt/trn_rl_repo')

import numpy as np

import concourse.bass as bass
import concourse.mybir as mybir
import concourse.tile as tile
from concourse import bacc

F32 = mybir.dt.float32
F16 = mybir.dt.float16
F8E4 = mybir.dt.float8e4
DR = mybir.MatmulPerfMode.DoubleRow
Exp = mybir.ActivationFunctionType.Exp
MULT = mybir.AluOpType.mult
ADD = mybir.AluOpType.add

B, S, D = 4, 2048, 1024
H, HD = 16, 64
N_CORES = 8
NSK = S // 128            # 16 k-chunks of 128
NQT = S // 128            # 16 q-tiles of 128
SCALE = 0.125             # 1/sqrt(HD)


def build_nc(skip_tail=False, skip_exp=False, sc_bufs=2, skip_v=False, fake_in=False):
    nc = bacc.Bacc(None, target_bir_lowering=False)

    x16_d = nc.dram_tensor("x16", [128, 8, S], F16, kind="ExternalInput")
    wqk_d = nc.dram_tensor("wqk", [128, 8, 1024], F16, kind="ExternalInput")
    wv_d = nc.dram_tensor("wv", [128, 8, 512], F16, kind="ExternalInput")
    wout_d = nc.dram_tensor("wout", [128, 4, D], F16, kind="ExternalInput")
    bqk_d = nc.dram_tensor("bqk", [128, 8], F32, kind="ExternalInput")
    bv_d = nc.dram_tensor("bv", [128, 8, HD], F16, kind="ExternalInput")
    ident_d = nc.dram_tensor("ident", [128, 128], F16, kind="ExternalInput")
    out_d = nc.dram_tensor("out", [S, D], F16, kind="ExternalOutput")

    with tile.TileContext(nc) as tc:
        with (
            tc.tile_pool(name="const", bufs=1) as cpool,
            tc.tile_pool(name="expT", bufs=4) as expT_pool,
            tc.tile_pool(name="ctxN", bufs=4) as ctxN_pool,
            tc.tile_pool(name="rcp", bufs=4) as rcp_pool,
            tc.tile_pool(name="osb", bufs=2) as out_pool,
            tc.tile_pool(name="scps", bufs=sc_bufs, space="PSUM") as sc_ps,
            tc.tile_pool(name="ctxps", bufs=2, space="PSUM") as ctx_ps,
            tc.tile_pool(name="shps", bufs=2, space="PSUM") as sh_ps,
        ):
            # ---- constants / persistent tiles (DMAs ordered so the
            # prologue's pair-0 stage1 work can start immediately) ----
            x16 = cpool.tile([128, 8, S], F16)
            wqk = cpool.tile([128, 8, 1024], F16)
            bqk = cpool.tile([128, 8], F32)
            wv = cpool.tile([128, 8, 512], F16)
            wout = cpool.tile([128, 4, D], F16)
            bv = cpool.tile([128, 8, HD], F16)
            ident = cpool.tile([128, 128], F16)
            nc.sync.dma_start(bqk[:], bqk_d[:])
            # pair-0 k then q weight slices, then x chunk 0
            nc.sync.dma_start(wqk[:, :, 512:640], wqk_d[:, :, 512:640])
            nc.sync.dma_start(wqk[:, :, 0:128], wqk_d[:, :, 0:128])
            nc.sync.dma_start(x16[:, :, 0:512], x16_d[:, :, 0:512])
            for n in range(1, 4):
                nc.sync.dma_start(x16[:, :, 512 * n:512 * (n + 1)],
                                  x16_d[:, :, 512 * n:512 * (n + 1)])
            nc.sync.dma_start(wv[:], wv_d[:])
            nc.sync.dma_start(bv[:], bv_d[:])
            nc.sync.dma_start(ident[:], ident_d[:])
            for p_ in range(1, 4):
                nc.sync.dma_start(wqk[:, :, 512 + 128 * p_:640 + 128 * p_],
                                  wqk_d[:, :, 512 + 128 * p_:640 + 128 * p_])
                nc.sync.dma_start(wqk[:, :, 128 * p_:128 * (p_ + 1)],
                                  wqk_d[:, :, 128 * p_:128 * (p_ + 1)])
            nc.sync.dma_start(wout[:], wout_d[:])

            # V': [s%128, sk, head, hd+1]; [..,64] = 1.0 for denominators
            v_sb = cpool.tile([128, NSK, 8, HD + 1], F16)
            nc.vector.memset(v_sb[:, :, :, HD], 1.0)

            # q/k fp8 double-buffered (ping-pong by pair parity).
            # dims: [part(=hd within head pair), qk, i(double-row), s]
            qk8 = [cpool.tile([128, 2, 2, S], F8E4, name=f"qk8_{b_}")
                   for b_ in range(2)]
            for b_ in range(2):
                nc.vector.memset(qk8[b_][:, :, 1, :], 0.0)

            # ctx^T accumulator [d-part packed by pair, pair, q] fp16
            ctxT = cpool.tile([128, 4, S], F16)

            # ---- stage1 helpers ----
            def s1_qk_tile(p, j, n):
                """pair p, j=0 q / j=1 k, seq chunk n -> qk8[p%2][:, j, 0, ...]."""
                ps = sh_ps.tile([128, 512], F32, name="s1", tag="sh")
                foff = 128 * p + 512 * j
                for kc in range(8):
                    nc.tensor.matmul(
                        ps[:], wqk[:, kc, foff:foff + 128],
                        x16[:, kc, 512 * n:512 * (n + 1)],
                        start=(kc == 0), stop=(kc == 7))
                nc.vector.tensor_scalar_add(
                    qk8[p % 2][:, j, 0, 512 * n:512 * (n + 1)], ps[:],
                    bqk[:, 4 * j + p:4 * j + p + 1])

            def s1_v_tile(p, t):
                """v for pair p (128 feats), seq tile t (128 rows)."""
                ps = sh_ps.tile([128, 128], F32, name="s1v", tag="sh")
                for kc in range(8):
                    nc.tensor.matmul(
                        ps[:], x16[:, kc, 128 * t:128 * (t + 1)],
                        wv[:, kc, 128 * p:128 * (p + 1)],
                        start=(kc == 0), stop=(kc == 7))
                nc.vector.tensor_copy(
                    v_sb[:, t, 2 * p:2 * p + 2, 0:HD],
                    ps.rearrange("a (h d) -> a h d", h=2))

            # ---- deferred PE work queue: popped between score/exp pairs so
            # the ACT engine (bottleneck) never starves while PE does the
            # ctx/transpose/stage1/outproj work of earlier units ----
            from collections import deque
            work_q = deque()

            def pop_work(k):
                for _ in range(k):
                    if work_q:
                        work_q.popleft()()

            def push_tail_hi(p, qc, hi, expTs, cns):
                """Queue head hi's ctx chains; after hi=1 also transposes
                and (for the last pair) this q-chunk's out-projection."""
                if skip_tail:
                    return
                def chain(hi, qt, cn):
                    def run():
                        h = 2 * p + hi
                        cps = ctx_ps.tile([128, HD + 1], F32,
                                          name="ctx", tag="ctx")
                        for sk in range(NSK):
                            nc.tensor.matmul(
                                cps[:],
                                expTs[hi][:, sk, 128 * qt:128 * (qt + 1)],
                                v_sb[:, sk, h, :],
                                start=(sk == 0), stop=(sk == NSK - 1))
                        rcp = rcp_pool.tile([128, 1], F32, name="rc",
                                            tag="rc")
                        nc.vector.reciprocal_approx_fast(
                            rcp[:], cps[:, HD:HD + 1])
                        nc.vector.scalar_tensor_tensor(
                            cn[:, hi, :], cps[:, 0:HD], rcp[:], bv[:, h, :],
                            op0=MULT, op1=ADD)
                    return run

                def transp(qt, cn):
                    def run():
                        tp = sh_ps.tile([128, 128], F16, name="tp", tag="sh")
                        nc.tensor.matmul(
                            tp[0:64, :], cn[:, 0, :], ident[:],
                            start=True, stop=True, is_transpose=True,
                            tile_position=(0, 0))
                        nc.tensor.matmul(
                            tp[64:128, :], cn[:, 1, :], ident[:],
                            start=True, stop=True, is_transpose=True,
                            tile_position=(0, 64))
                        qoff = 512 * qc + 128 * qt
                        nc.vector.tensor_copy(
                            ctxT[:, p, qoff:qoff + 128], tp[:])
                    return run

                for qt in range(4):
                    work_q.append(chain(hi, qt, cns[qt]))
                    if hi == 1:
                        work_q.append(transp(qt, cns[qt]))

                if p == 3 and hi == 1:
                    def oproj(qt, dc):
                        def run():
                            qoff = 512 * qc + 128 * qt
                            ops = sh_ps.tile([128, 512], F32, name="op",
                                             tag="sh")
                            for c in range(4):
                                nc.tensor.matmul(
                                    ops[:], ctxT[:, c, qoff:qoff + 128],
                                    wout[:, c, 512 * dc:512 * (dc + 1)],
                                    start=(c == 0), stop=(c == 3))
                            o16 = out_pool.tile([128, 512], F16, name="o")
                            nc.vector.tensor_copy(o16[:], ops[:])
                            nc.sync.dma_start(
                                out_d[qoff:qoff + 128,
                                      512 * dc:512 * (dc + 1)], o16[:])
                        return run
                    for qt in range(4):
                        for dc in range(2):
                            work_q.append(oproj(qt, dc))

            # ---- prologue: just k chunk 0 + q chunk 0 of pair 0; the rest
            # of pair-0 stage1 and its v-proj ride the work queue ----
            s1_qk_tile(0, 1, 0)
            s1_qk_tile(0, 0, 0)
            for (j, n) in [(1, 1), (1, 2), (1, 3), (0, 1), (0, 2), (0, 3)]:
                work_q.append((lambda j=j, n=n: s1_qk_tile(0, j, n)))
            if not skip_v:
                for t in range(NSK):
                    work_q.append((lambda t=t: s1_v_tile(0, t)))

            # ---- main software-pipelined unit loop ----
            for p in range(4):
                for qc in range(4):
                    u = 4 * p + qc
                    buf = qk8[p % 2]
                    expTs = {}
                    cns = [ctxN_pool.tile([128, 2, HD], F16, name="cnq",
                                          tag="cn") for _ in range(4)]
                    slot = 0
                    for hi in range(2):
                        expTs[hi] = expT_pool.tile([128, NSK, 512], F16,
                                                   name=f"e{hi}", tag="expT")
                        for g in range(8):
                            scp = sc_ps.tile([128, 2, 512], F32, name="sc",
                                             tag="sc")
                            for gg in range(2):
                                sk = 2 * g + gg
                                nc.tensor.matmul(
                                    scp[:, gg, :],
                                    buf[64 * hi:64 * (hi + 1), 1, :,
                                        128 * sk:128 * (sk + 1)],
                                    buf[64 * hi:64 * (hi + 1), 0, :,
                                        512 * qc:512 * (qc + 1)],
                                    start=True, stop=True, perf_mode=DR)
                            if not skip_exp:
                                nc.scalar.activation(
                                    expTs[hi][:, 2 * g:2 * g + 2, :], scp[:],
                                    Exp, scale=SCALE)
                            else:
                                nc.vector.tensor_copy(
                                    expTs[hi][:, 2 * g:2 * g + 2, :], scp[:])
                            pop_work(2 if slot % 3 == 2 else 1)
                            slot += 1
                        push_tail_hi(p, qc, hi, expTs, cns)

                    # queue work for later units
                    if p < 3 and not skip_v:
                        for t in range(4):
                            work_q.append(
                                (lambda p=p, t=t, qc=qc:
                                 s1_v_tile(p + 1, 4 * qc + t)))
                        jn = [(1, 0), (1, 1)] if qc == 0 else \
                             [(1, 2), (1, 3)] if qc == 1 else \
                             [(0, 0), (0, 1)] if qc == 2 else \
                             [(0, 2), (0, 3)]
                        for (j, n) in jn:
                            work_q.append(
                                (lambda p=p, j=j, n=n:
                                 s1_qk_tile(p + 1, j, n)))

            # drain
            while work_q:
                work_q.popleft()()

    nc.compile()
    return nc


# ---------------------------------------------------------------------------
# host side: shard, run SPMD, gather
# ---------------------------------------------------------------------------

_RUNNER = None


def _make_runner(nc, n_cores):
    """Jit-once SPMD runner via PJRT (axon)."""
    import jax
    from jax.sharding import Mesh, PartitionSpec
    from jax.experimental.shard_map import shard_map
    from concourse import bass2jax
    from concourse.bass2jax import _bass_exec_p, install_neuronx_cc_hook

    install_neuronx_cc_hook()
    partition_name = nc.partition_id_tensor.name if nc.partition_id_tensor else None

    in_names, out_names, out_avals, zero_outs = [], [], [], []
    for alloc in nc.m.functions[0].allocations:
        if not isinstance(alloc, mybir.MemoryLocationSet):
            continue
        name = alloc.memorylocations[0].name
        if alloc.kind == "ExternalInput":
            if name != partition_name:
                in_names.append(name)
        elif alloc.kind == "ExternalOutput":
            out_names.append(name)
            shape = tuple(alloc.tensor_shape)
            dtype = mybir.dt.np(alloc.dtype)
            out_avals.append(jax.core.ShapedArray(shape, dtype))
            zero_outs.append(np.zeros(shape, dtype))
    n_params = len(in_names)
    n_outs = len(out_avals)
    all_in_names = list(in_names) + list(out_names)
    if partition_name is not None:
        all_in_names.append(partition_name)

    def _body(*args):
        operands = list(args)
        if partition_name is not None:
            operands.append(bass2jax.partition_id_tensor())
        outs = _bass_exec_p.bind(
            *operands,
            out_avals=tuple(out_avals),
            in_names=tuple(all_in_names),
            out_names=tuple(out_names),
            lowering_input_output_aliases=(),
            sim_require_finite=True,
            sim_require_nnan=True,
            nc=nc,
        )
        return tuple(outs)

    devices = jax.devices()[:n_cores]
    mesh = Mesh(np.asarray(devices), ("core",))
    in_specs = (PartitionSpec("core"),) * (n_params + n_outs)
    out_specs = (PartitionSpec("core"),) * n_outs
    jitted = jax.jit(
        shard_map(_body, mesh=mesh, in_specs=in_specs, out_specs=out_specs,
                  check_rep=False),
        keep_unused=True,
    )

    def run(in_maps):
        concat_in = [
            np.concatenate([np.asarray(in_maps[c][n]) for c in range(n_cores)],
                           axis=0)
            for n in in_names
        ]
        concat_zero = [
            np.zeros((n_cores * z.shape[0], *z.shape[1:]), z.dtype)
            for z in zero_outs
        ]
        out_arrs = jitted(*concat_in, *concat_zero)
        jax.block_until_ready(out_arrs)
        return [
            {n: np.asarray(out_arrs[i]).reshape(n_cores, *out_avals[i].shape)[c]
             for i, n in enumerate(out_names)}
            for c in range(n_cores)
        ]

    return run


def _shard_inputs(qkv, W_in, b_in, W_out, b_out):
    """Build the 8 per-core input dicts."""
    f16 = np.float16
    x = np.asarray(qkv, np.float32)
    W_in = np.asarray(W_in, np.float32)
    b_in = np.asarray(b_in, np.float32)
    W_out = np.asarray(W_out, np.float32)
    ident = np.eye(128, dtype=f16)

    in_maps = []
    for c in range(N_CORES):
        b, g = divmod(c, 2)
        qs = slice(512 * g, 512 * (g + 1))
        ks = slice(1024 + 512 * g, 1024 + 512 * (g + 1))
        vs = slice(2048 + 512 * g, 2048 + 512 * (g + 1))
        xT = np.ascontiguousarray(x[b].T)                     # [D, S]
        # x16[p, kc, s] = xT[128*kc+p, s]
        x16 = xT.reshape(8, 128, S).transpose(1, 0, 2).astype(f16)
        # wqk[p, kc, f]: f 0..511 q feats, 512..1023 k feats
        wq = W_in[:, qs].reshape(8, 128, 512).transpose(1, 0, 2)
        wk = W_in[:, ks].reshape(8, 128, 512).transpose(1, 0, 2)
        wqk = np.concatenate([wq, wk], axis=2).astype(f16)
        wv = W_in[:, vs].reshape(8, 128, 512).transpose(1, 0, 2).astype(f16)
        # wout[p, c_, dout] = W_out[512*g + 128*c_ + p, dout]
        wout = W_out[512 * g:512 * (g + 1), :].reshape(4, 128, D) \
            .transpose(1, 0, 2).astype(f16)
        bqk = np.concatenate([b_in[qs], b_in[ks]]).reshape(8, 128).T \
            .astype(np.float32)
        bqk = np.ascontiguousarray(bqk)
        bv = np.broadcast_to(b_in[vs].reshape(8, HD), (128, 8, HD)) \
            .astype(f16)
        in_maps.append({
            "x16": x16,
            "wqk": wqk,
            "wv": wv,
            "wout": wout,
            "bqk": bqk,
            "bv": np.ascontiguousarray(bv),
            "ident": ident,
        })
    return in_maps


def kernel(qkv, W_in, b_in, W_out, b_out):
    global _RUNNER
    if _RUNNER is None:
        nc = build_nc()
        _RUNNER = _make_runner(nc, N_CORES)
    in_maps = _shard_inputs(qkv, W_in, b_in, W_out, b_out)
    results = _RUNNER(in_maps)
    b_out = np.asarray(b_out, np.float32)
    out = np.empty((B, S, D), np.float32)
    for b in range(B):
        out[b] = (results[2 * b]["out"].astype(np.float32)
                  + results[2 * b + 1]["out"].astype(np.float32) + b_out)
    return out


if __name__ == "__main__":
    rng = np.random.default_rng(0)
    qkv = rng.standard_normal((B, S, D)).astype(np.float32)
    sc = 1.0 / np.sqrt(D)
    W_in = rng.uniform(-sc, sc, (D, 3 * D)).astype(np.float32)
    b_in = rng.uniform(-sc, sc, (3 * D,)).astype(np.float32)
    W_out = rng.uniform(-sc, sc, (D, D)).astype(np.float32)
    b_out = rng.uniform(-sc, sc, (D,)).astype(np.float32)
    got = kernel(qkv, W_in, b_in, W_out, b_out)
    print("kernel ran, output shape", got.shape)
